# revision 2
# baseline (speedup 1.0000x reference)
"""Trainium2 Bass/Tile kernel for nn_CrossAttentionFiLM — factorized attention, v5.

See kernel2/kernel3 for the factorization derivation (query length 1 =>
k/v never materialized; Wk folded into q, Wv applied after the
attention-weighted sum of tf).  v5 restructures for DMA overlap:

- wl/wq weight chunks are DMA'd BEFORE the tf block prefetch so phase0's
  matmul chain is not starved behind 12.6MB of tf traffic (SP queue is FIFO).
- LN1's affine is folded into the FiLM weights on the host (exact):
      out1 = LN0(h1) * gammaG + betaG,
      gammaG = z@(diag(g1)Wg).T + g1(1+bg)
      betaG  = z@(Wb + diag(b1)Wg).T + (bb + b1(1+bg))
  so LN1 on-chip is bare normalization and needs no broadcast g/b rows.
- Projection biases (gammaG/betaG/bl/bq'/bo rows) enter as rank-1 PE
  matmuls (ones[1,BC] x biasrow[1,512]) accumulated into the same PSUM
  group — replaces per-row [128,F] broadcast DMAs (0.5MB each).
- qk evacuations batched 3 cc-chunks per cast (16 casts instead of 48).
- wo / wvT loads deferred into the main loop's DMA shadow.
"""

import os
import sys

for _p in ("/opt/trn_rl_repo",):
    if os.path.isdir(_p) and _p not in sys.path:
        sys.path.append(_p)

import numpy as np
import ml_dtypes

os.environ.setdefault("JAX_COMPILATION_CACHE_DIR", "/tmp/jax_comp_cache")

import concourse.bass as bass
import concourse.tile as tile
from concourse import bacc, mybir
from concourse.bass_utils import run_bass_kernel_spmd
from concourse.masks import make_identity

BF16 = mybir.dt.bfloat16
F32 = mybir.dt.float32
AF = mybir.ActivationFunctionType
ALU = mybir.AluOpType

B, T, F_DIM, Z_DIM, TXT_DIM, H = 1024, 128, 1024, 256, 768, 8
D = F_DIM // H
NCORES = 8
BC = B // NCORES
EPS = 1e-5
CC_Z = Z_DIM // 128
CC_F = F_DIM // 128
CC_C = TXT_DIM // 128
M0 = 16.0

BBLK = 16
NBLK = BC // BBLK
HB = 8
N_HB = BC // HB

# rank-1 bias rows (bf16, applied on PE): gammaG, betaG, bl, bq', bo
RB_G, RB_B, RB_L, RB_Q, RB_O = range(5)
# broadcast f32 rows (tail only): bv, ln2_g, ln2_b
IB_BV, IB_G2, IB_B2 = range(3)


def build(nc):
    xt = nc.dram_tensor("xt", [128, CC_F, BC], BF16, kind="ExternalInput").ap()
    zt = nc.dram_tensor("zt", [128, CC_Z, BC], BF16, kind="ExternalInput").ap()
    tfT = nc.dram_tensor("tfT", [CC_C, 128, BC, T], BF16,
                         kind="ExternalInput").ap()
    tf2d = nc.dram_tensor("tf2d", [T, BC, TXT_DIM], BF16,
                          kind="ExternalInput").ap()
    maskb = nc.dram_tensor("maskb", [T, BC], F32, kind="ExternalInput").ap()
    wg = nc.dram_tensor("wg", [128, CC_Z, F_DIM], BF16, kind="ExternalInput").ap()
    wb = nc.dram_tensor("wb", [128, CC_Z, F_DIM], BF16, kind="ExternalInput").ap()
    wl = nc.dram_tensor("wl", [CC_F, 128, F_DIM], BF16, kind="ExternalInput").ap()
    wq = nc.dram_tensor("wq", [CC_F, 128, F_DIM], BF16, kind="ExternalInput").ap()
    wo = nc.dram_tensor("wo", [CC_F, 128, F_DIM], BF16, kind="ExternalInput").ap()
    wkT = nc.dram_tensor("wkT", [128, H, TXT_DIM], BF16,
                         kind="ExternalInput").ap()
    wvT = nc.dram_tensor("wvT", [128, CC_C, H, D], BF16,
                         kind="ExternalInput").ap()
    biasb = nc.dram_tensor("biasb", [5, F_DIM], BF16, kind="ExternalInput").ap()
    bias = nc.dram_tensor("bias", [3, F_DIM], F32, kind="ExternalInput").ap()
    out = nc.dram_tensor("out", [BC, F_DIM], F32, kind="ExternalOutput").ap()

    with tile.TileContext(nc) as tc:
        _emit(nc, tc, xt, zt, tfT, tf2d, maskb, wg, wb, wl, wq, wo, wkT, wvT,
              biasb, bias, out)
    return nc


def _emit(nc, tc, xt, zt, tfT, tf2d, maskb, wg, wb, wl, wq, wo, wkT, wvT,
          biasb, bias, out):
    from contextlib import ExitStack

    ctxmgr = ExitStack()
    with ctxmgr:
        singles = ctxmgr.enter_context(tc.tile_pool(name="singles", bufs=1))
        scratch = ctxmgr.enter_context(tc.tile_pool(name="scratch", bufs=4))
        biasp = ctxmgr.enter_context(tc.tile_pool(name="biasp", bufs=2))
        wstream = ctxmgr.enter_context(tc.tile_pool(name="wstream", bufs=8))
        tftp = ctxmgr.enter_context(tc.tile_pool(name="tftp", bufs=2))
        tf2p = ctxmgr.enter_context(tc.tile_pool(name="tf2p", bufs=2))
        attp = ctxmgr.enter_context(tc.tile_pool(name="attp", bufs=2))
        ps_big = ctxmgr.enter_context(
            tc.tile_pool(name="ps_big", bufs=1, space="PSUM"))
        ps_den = ctxmgr.enter_context(
            tc.tile_pool(name="ps_den", bufs=1, space="PSUM"))

        # ---- phase0-critical resident loads (before any tf traffic) ----
        zt_sb = singles.tile([128, CC_Z, BC], BF16)
        nc.sync.dma_start(out=zt_sb, in_=zt)
        wg_sb = singles.tile([128, CC_Z, F_DIM], BF16)
        nc.sync.dma_start(out=wg_sb, in_=wg)
        wb_sb = singles.tile([128, CC_Z, F_DIM], BF16)
        nc.sync.dma_start(out=wb_sb, in_=wb)
        xt_sb = singles.tile([128, CC_F, BC], BF16)
        nc.sync.dma_start(out=xt_sb, in_=xt)
        biasb_sb = singles.tile([1, 5, F_DIM], BF16)
        nc.sync.dma_start(out=biasb_sb,
                          in_=biasb.rearrange("r f -> (r f)")[None, :])
        maskb_sb = singles.tile([T, BC], F32)
        nc.sync.dma_start(out=maskb_sb, in_=maskb)

        wl_t = [wstream.tile([128, F_DIM], BF16, tag="w", name=f"wl{i}")
                for i in range(CC_F)]
        for cc in range(CC_F):
            nc.sync.dma_start(out=wl_t[cc], in_=wl[cc])
        wkT_sb = singles.tile([128, H, TXT_DIM], BF16)
        nc.sync.dma_start(out=wkT_sb, in_=wkT)

        ident = singles.tile([128, 128], F32)
        make_identity(nc, ident)
        eps_t = singles.tile([128, 1], F32)
        nc.vector.memset(eps_t, EPS)
        ones_bf = singles.tile([T, 1], BF16)
        nc.vector.memset(ones_bf, 1.0)
        ones_row = singles.tile([1, BC], BF16)
        nc.vector.memset(ones_row, 1.0)

        def bias_row(i):
            bt = biasp.tile([128, F_DIM], F32, tag="bias")
            row = bias[i]
            src = bass.AP(tensor=row.tensor, offset=row.offset,
                          ap=[[0, 128]] + list(row.ap))
            nc.sync.dma_start(out=bt, in_=src)
            return bt

        def rank1_bias(ps_t, row, stop=True):
            """ps_t[:, :] += ones[b] * biasb[row, f] via two rank-1 matmuls."""
            for nh in range(2):
                nc.tensor.matmul(
                    ps_t[:, nh * 512:(nh + 1) * 512],
                    lhsT=ones_row,
                    rhs=biasb_sb[:, row, nh * 512:(nh + 1) * 512],
                    start=False, stop=stop)

        # persistent activations
        out1 = singles.tile([BC, F_DIM], F32)
        out1t = singles.tile([128, CC_F, BC], BF16)
        qT = singles.tile([128, H, BC], BF16)
        qkT_sb = singles.tile([128, CC_C, BC, H], BF16)
        ctxT_sb = singles.tile([128, CC_C, BC, H], BF16)
        rden = singles.tile([BC, H], F32)
        denT_ps = ps_den.tile([H, BC], F32, tag="den")

        def layer_norm_raw(dst, src):
            """dst = (src - mean) * rsqrt(var + eps); no affine."""
            lnw = scratch.tile([BC, 16], F32, tag="lnw")
            st = lnw[:, 0:12].rearrange("p (g s) -> p g s", g=2)
            mv = lnw[:, 12:14]
            sd = lnw[:, 14:15]
            rstd = lnw[:, 15:16]
            src3 = src.rearrange("p (g d) -> p g d", g=2)
            for sg in range(2):
                nc.vector.bn_stats(out=st[:, sg, :], in_=src3[:, sg, :])
            nc.vector.bn_aggr(out=mv, in_=st)
            nc.scalar.activation(out=sd, in_=mv[:, 1:2], func=AF.Sqrt,
                                 bias=eps_t, scale=1.0)
            nc.vector.reciprocal(out=rstd, in_=sd)
            nc.vector.tensor_scalar(out=dst, in0=src, scalar1=mv[:, 0:1],
                                    scalar2=rstd, op0=ALU.subtract,
                                    op1=ALU.mult)

        # ---- tf streaming: tfT per 16-row block, tf2 per 8-row half-block
        blk_tiles = []
        tf2_tiles = []

        def emit_blk_dma(blk):
            b0 = blk * BBLK
            tft = tftp.tile([128, CC_C, BBLK, T], BF16, tag="tft")
            src = tfT[:, :, b0:b0 + BBLK, :].rearrange("cc c b t -> c cc b t")
            nc.sync.dma_start(out=tft, in_=src)
            blk_tiles.append(tft)

        def emit_tf2_dma(hb):
            b0 = hb * HB
            tf2 = tf2p.tile([T, HB, CC_C, 128], BF16, tag="tf2")
            src2 = tf2d[:, b0:b0 + HB, :]
            nc.sync.dma_start(out=tf2, in_=src2)
            tf2_tiles.append(tf2)

        emit_blk_dma(0)
        emit_tf2_dma(0)
        emit_tf2_dma(1)

        # ---- phase0: FiLM params (LN1 affine folded in on host) ----
        gammaG = scratch.tile([BC, F_DIM], F32, tag="act")
        betaG = scratch.tile([BC, F_DIM], F32, tag="act")
        for w_sb, brow, dst in ((wg_sb, RB_G, gammaG), (wb_sb, RB_B, betaG)):
            ps_t = ps_big.tile([BC, F_DIM], F32, tag="big")
            for cc in range(CC_Z):
                for nh in range(2):
                    nc.tensor.matmul(
                        ps_t[:, nh * 512:(nh + 1) * 512],
                        lhsT=zt_sb[:, cc, :],
                        rhs=w_sb[:, cc, nh * 512:(nh + 1) * 512],
                        start=(cc == 0), stop=False)
            rank1_bias(ps_t, brow)
            nc.vector.tensor_copy(dst, ps_t)

        emit_blk_dma(1)
        emit_tf2_dma(2)

        # ---- h1 = x@Wl.T + bl (stays in PSUM); out1 = LN0(h1)*gG + bG ----
        h1_ps = ps_big.tile([BC, F_DIM], F32, tag="big")
        for cc in range(CC_F):
            for nh in range(2):
                nc.tensor.matmul(
                    h1_ps[:, nh * 512:(nh + 1) * 512],
                    lhsT=xt_sb[:, cc, :],
                    rhs=wl_t[cc][:, nh * 512:(nh + 1) * 512],
                    start=(cc == 0), stop=False)
        rank1_bias(h1_ps, RB_L)
        wq_t = [wstream.tile([128, F_DIM], BF16, tag="w", name=f"wq{i}")
                for i in range(CC_F)]
        for cc in range(CC_F):
            nc.sync.dma_start(out=wq_t[cc], in_=wq[cc])
        ln1 = scratch.tile([BC, F_DIM], F32, tag="act")
        layer_norm_raw(ln1, h1_ps)
        nc.vector.tensor_mul(out1, ln1, gammaG)
        nc.vector.tensor_add(out1, out1, betaG)

        # ---- q; qT; qk fold ----
        with tc.tile_pool(name="ps_tp", bufs=2, space="PSUM") as ps_tp:
            for cc in range(CC_F):
                tp = ps_tp.tile([128, 384], F32, tag="tp")
                nc.tensor.transpose(tp[:, 0:128],
                                    out1[:, cc * 128:(cc + 1) * 128], ident)
                nc.scalar.activation(out=out1t[:, cc, :], in_=tp[:, 0:128],
                                     func=AF.Copy)
            ps_t = ps_big.tile([BC, F_DIM], F32, tag="big")
            for cc in range(CC_F):
                for nh in range(2):
                    nc.tensor.matmul(
                        ps_t[:, nh * 512:(nh + 1) * 512],
                        lhsT=out1t[:, cc, :],
                        rhs=wq_t[cc][:, nh * 512:(nh + 1) * 512],
                        start=(cc == 0), stop=False)
            rank1_bias(ps_t, RB_Q)
            q_sb = scratch.tile([BC, F_DIM], F32, tag="act")
            nc.vector.tensor_copy(q_sb, ps_t)
            for h in range(H):
                tp = ps_tp.tile([128, 384], F32, tag="tp")
                nc.tensor.transpose(tp[:, 0:128],
                                    q_sb[:, h * 128:(h + 1) * 128], ident)
                nc.scalar.activation(out=qT[:, h, :], in_=tp[:, 0:128],
                                     func=AF.Copy)
            # qkT[c, cc, b, h]; 3 cc-chunks per PSUM tile -> batched evac
            for h in range(H):
                for g in range(2):
                    qk_ps = ps_tp.tile([128, 384], F32, tag="tp")
                    for i in range(3):
                        cc = g * 3 + i
                        nc.tensor.matmul(
                            qk_ps[:, i * 128:(i + 1) * 128],
                            lhsT=wkT_sb[:, h, cc * 128:(cc + 1) * 128],
                            rhs=qT[:, h, :], start=True, stop=True)
                    nc.vector.tensor_copy(
                        qkT_sb[:, g * 3:g * 3 + 3, :, h], qk_ps)

        # wo/wvT land during the loop's DMA shadow
        wo_t = [wstream.tile([128, F_DIM], BF16, tag="w", name=f"wo{i}")
                for i in range(CC_F)]
        for cc in range(CC_F):
            nc.sync.dma_start(out=wo_t[cc], in_=wo[cc])
        wvT_sb = singles.tile([128, CC_C, H, D], BF16)
        nc.sync.dma_start(out=wvT_sb, in_=wvT)

        # ---- main loop ----
        def emit_scores(blk, hb):
            tft = blk_tiles[blk]
            sc = ps_sc.tile([T, HB, H], F32, tag="sc")
            att_t = attp.tile([T, HB, H], BF16, tag="att")
            for j in range(HB):
                b = hb * HB + j
                bj = b - blk * BBLK
                for cc in range(CC_C):
                    nc.tensor.matmul(
                        sc[:, j, :],
                        lhsT=tft[:, cc, bj, :],
                        rhs=qkT_sb[:, cc, b, :],
                        start=(cc == 0), stop=(cc == CC_C - 1))
            for j in range(HB):
                b = hb * HB + j
                nc.scalar.activation(out=att_t[:, j, :], in_=sc[:, j, :],
                                     func=AF.Exp, bias=maskb_sb[:, b:b + 1],
                                     scale=1.0)
            return att_t

        def emit_ctx(blk, hb, att_t):
            tf2 = tf2_tiles[hb]
            cxp = ps_cx.tile([128, CC_C, HB, H], F32, tag="cx")
            for j in range(HB):
                b = hb * HB + j
                bj = b - blk * BBLK
                nc.tensor.matmul(denT_ps[:, b:b + 1], lhsT=att_t[:, j, :],
                                 rhs=ones_bf, start=True, stop=True)
                for cc in range(CC_C):
                    nc.tensor.matmul(
                        cxp[:, cc, j, :],
                        lhsT=tf2[:, j, cc, :],
                        rhs=att_t[:, j, :], start=True, stop=True)
            b0 = hb * HB
            nc.vector.tensor_copy(ctxT_sb[:, :, b0:b0 + HB, :], cxp)

        ctxf_ps = ps_big.tile([BC, F_DIM], F32, tag="big")

        def stage_b_half(half):
            p0 = half * 64
            for h in range(H):
                for cc in range(CC_C):
                    nc.tensor.matmul(
                        ctxf_ps[p0:p0 + 64, h * D:(h + 1) * D],
                        lhsT=ctxT_sb[:, cc, p0:p0 + 64, h],
                        rhs=wvT_sb[:, cc, h, :],
                        start=(cc == 0), stop=(cc == CC_C - 1))

        with tc.tile_pool(name="ps_sc", bufs=2, space="PSUM") as ps_sc, \
             tc.tile_pool(name="ps_cx", bufs=2, space="PSUM") as ps_cx:
            pending = None
            for hb in range(N_HB):
                blk = hb * HB // BBLK
                att_t = emit_scores(blk, hb)
                if pending is not None:
                    emit_ctx(*pending)
                    if pending[1] == 7:
                        stage_b_half(0)
                pending = (blk, hb, att_t)
                if len(tf2_tiles) <= hb + 2 and hb + 2 < N_HB:
                    emit_tf2_dma(hb + 2)
                if blk + 1 < NBLK and len(blk_tiles) <= blk + 1:
                    emit_blk_dma(blk + 1)
            emit_ctx(*pending)
            stage_b_half(1)

        # ---- tail ----
        denT_sb = scratch.tile([H, BC], F32, tag="lnw")
        nc.vector.tensor_copy(denT_sb, denT_ps)
        with tc.tile_pool(name="ps_tl", bufs=2, space="PSUM") as ps_tp:
            den_ps = ps_tp.tile([BC, 384], F32, tag="tp")
            nc.tensor.transpose(den_ps[:, 0:H], denT_sb, ident[0:H, 0:H])
            nc.vector.reciprocal(out=rden, in_=den_ps[:, 0:H])

            ps_t = ctxf_ps
            ctxf = scratch.tile([BC, F_DIM], F32, tag="act")
            bv_t = bias_row(IB_BV)
            for h in range(H):
                hs = slice(h * D, (h + 1) * D)
                nc.vector.scalar_tensor_tensor(
                    out=ctxf[:, hs], in0=ps_t[:, hs],
                    scalar=rden[:, h:h + 1], in1=bv_t[:, hs],
                    op0=ALU.mult, op1=ALU.add)
            ctxfT = singles.tile([128, H, BC], BF16)
            for h in range(H):
                tp = ps_tp.tile([BC, 384], F32, tag="tp")
                nc.tensor.transpose(tp[:, 0:128], ctxf[:, h * D:(h + 1) * D],
                                    ident)
                nc.scalar.activation(out=ctxfT[:, h, :], in_=tp[:, 0:128],
                                     func=AF.Copy)

            ao_ps = ps_big.tile([BC, F_DIM], F32, tag="big")
            for cc in range(CC_F):
                for nh in range(2):
                    nc.tensor.matmul(
                        ao_ps[:, nh * 512:(nh + 1) * 512],
                        lhsT=ctxfT[:, cc, :],
                        rhs=wo_t[cc][:, nh * 512:(nh + 1) * 512],
                        start=(cc == 0), stop=False)
            rank1_bias(ao_ps, RB_O)
            ln2 = scratch.tile([BC, F_DIM], F32, tag="act")
            layer_norm_raw(ln2, ao_ps)
            # y = ln2*g2 + (out1 + b2); gelu
            ob2 = scratch.tile([BC, F_DIM], F32, tag="act")
            nc.vector.tensor_add(ob2, out1, bias_row(IB_B2))
            y = scratch.tile([BC, F_DIM], F32, tag="act")
            nc.vector.tensor_mul(y, ln2, bias_row(IB_G2))
            nc.vector.tensor_add(y, y, ob2)
            out_sb = scratch.tile([BC, F_DIM], F32, tag="act")
            nc.scalar.activation(out=out_sb, in_=y, func=AF.Gelu)
            nc.sync.dma_start(out=out, in_=out_sb)


def _chunk_weight(w, n_cc, scale=None, dtype=np.float32, chunk_major=False):
    wt = np.asarray(w, np.float32).T
    if scale is not None:
        wt = wt * scale
    c_in, f_out = wt.shape
    assert c_in == n_cc * 128
    a = wt.reshape(n_cc, 128, f_out)
    if not chunk_major:
        a = a.transpose(1, 0, 2)
    return np.ascontiguousarray(a.astype(dtype))


def prep_inputs(x, z, text_feat, attention, Wg, bg, Wb, bb, Wl, bl, ln1_g,
                ln1_b, Wq, bq, Wk, bk, Wv, bv, Wo, bo, ln2_g, ln2_b):
    f32 = np.float32
    bf16 = ml_dtypes.bfloat16
    x = np.asarray(x, f32)
    z = np.asarray(z, f32)
    text_feat = np.asarray(text_feat, f32)
    attention = np.asarray(attention, np.int32)
    g1 = np.asarray(ln1_g, f32)
    b1 = np.asarray(ln1_b, f32)
    bg = np.asarray(bg, f32)
    bb = np.asarray(bb, f32)
    Wg = np.asarray(Wg, f32)
    Wb = np.asarray(Wb, f32)

    # fold LN1 affine into the FiLM branch (exact):
    #   out1 = LN0(h1) * gammaG + betaG
    WgG = g1[:, None] * Wg
    WbG = Wb + b1[:, None] * Wg
    rowG = g1 * (1.0 + bg)
    rowB = bb + b1 * (1.0 + bg)

    xt = np.ascontiguousarray(
        x.reshape(NCORES, BC, CC_F, 128).transpose(0, 3, 2, 1).astype(bf16))
    zt = np.ascontiguousarray(
        z.reshape(NCORES, BC, CC_Z, 128).transpose(0, 3, 2, 1).astype(bf16))
    tfb = text_feat.astype(bf16).reshape(NCORES, BC, T, CC_C, 128)
    tfT = np.ascontiguousarray(tfb.transpose(0, 3, 4, 1, 2))
    maskb = np.ascontiguousarray(
        (-1e30 * (attention.reshape(NCORES, BC, T) == 0) - M0)
        .transpose(0, 2, 1).astype(f32))

    shared = {
        "wg": _chunk_weight(WgG, CC_Z, dtype=bf16),
        "wb": _chunk_weight(WbG, CC_Z, dtype=bf16),
        "wl": _chunk_weight(Wl, CC_F, dtype=bf16, chunk_major=True),
        "wq": _chunk_weight(Wq, CC_F, scale=1.0 / np.sqrt(D), dtype=bf16,
                            chunk_major=True),
        "wo": _chunk_weight(Wo, CC_F, dtype=bf16, chunk_major=True),
        "wkT": np.ascontiguousarray(
            np.asarray(Wk, f32).reshape(H, D, TXT_DIM).transpose(1, 0, 2)
            .astype(bf16)),
        "wvT": np.ascontiguousarray(
            np.asarray(Wv, f32).T.reshape(CC_C, 128, H, D).transpose(1, 0, 2, 3)
            .astype(bf16)),
        "biasb": np.ascontiguousarray(np.stack([
            rowG,
            rowB,
            np.asarray(bl, f32),
            np.asarray(bq, f32) / np.sqrt(D),
            np.asarray(bo, f32),
        ]).astype(bf16)),
        "bias": np.ascontiguousarray(np.stack([
            np.asarray(bv, f32),
            np.asarray(ln2_g, f32),
            np.asarray(ln2_b, f32),
        ]).astype(f32)),
    }
    in_maps = []
    for c in range(NCORES):
        m = dict(shared)
        m["xt"] = xt[c]
        m["zt"] = zt[c]
        m["tfT"] = tfT[c]
        m["tf2d"] = np.ascontiguousarray(
            tfb[c].reshape(BC, T, TXT_DIM).transpose(1, 0, 2))
        m["maskb"] = maskb[c]
        in_maps.append(m)
    return in_maps


_CACHE = {}


def get_compiled():
    if "nc" not in _CACHE:
        nc = bacc.Bacc("TRN2", target_bir_lowering=False, debug=False,
                       enable_asserts=False)
        build(nc)
        nc.compile()
        _CACHE["nc"] = nc
    return _CACHE["nc"]


def run(in_maps, trace=False, **kw):
    nc = get_compiled()
    return run_bass_kernel_spmd(nc, in_maps, list(range(NCORES)), trace=trace,
                                **kw)


def kernel(**inputs):
    in_maps = prep_inputs(**inputs)
    res = run(in_maps)
    out = np.concatenate([res.results[c]["out"] for c in range(NCORES)],
                         axis=0)
    return np.ascontiguousarray(out.astype(np.float32))


if __name__ == "__main__":
    print("building + compiling...")
    get_compiled()
    print("done")
